# revision 16
# baseline (speedup 1.0000x reference)
"""Trainium2 Bass kernel for nn_Encoder_88235808129468 (scatter_memory).

reference semantics:
    proj = relu(emb @ W + b)                      # [B, N, 32]
    proj *= (n < entity_num[b])                   # mask padded entities
    out[b, :, y, x] += proj[b, n, :]              # scatter-add into [B, 32, H, W]

Strategy (pure data-parallel over batch: 8 cores x 8 batches):
  - ExternalOutput DRAM buffers are pre-zeroed by the PJRT runner (documented
    contract in bass2jax.run_bass_via_pjrt), so the kernel only writes the
    scattered entity rows; untouched cells stay zero.
  - Device output layout is HW-major [25600, 32] rows per batch; the host
    reorders axes to [B, 32, H, W] at the end.
  - Scatter-add duplicates are resolved with a 512x512 selection matrix
    (idx_i == idx_j, with the padding mask folded in): a matmul against proj
    gives EVERY entity the full group total for its cell, then an overwrite
    indirect-DMA scatter writes the rows. Colliding writes carry identical
    bytes, so write order does not matter.
  - proj and the group totals are computed in transposed [32, N] space so
    the matmuls run at F=512 with few instructions, then PE-transposed back
    to entity-major tiles for the scatter.
"""
import sys
import types

sys.path.insert(0, "/opt/trn_rl_repo")

import numpy as np


def _install_axon_hooks_stub():
    """bass_utils imports antenv.axon_hooks when tracing; give it a no-op."""
    if "antenv.axon_hooks" in sys.modules:
        return
    mod = types.ModuleType("antenv.axon_hooks")
    _state = {"hook": None}
    mod.set_axon_ntff_profile_hook = lambda h: _state.__setitem__("hook", h)
    mod.get_axon_ntff_profile_hook = lambda: _state["hook"]
    sys.modules["antenv.axon_hooks"] = mod


_install_axon_hooks_stub()

from concourse import bass, mybir, bacc  # noqa: E402
from concourse.bass_utils import run_bass_kernel_spmd  # noqa: E402
import concourse.tile as tile  # noqa: E402

# Problem constants (hardcoded per harness contract)
B, N, D_IN, D_OUT = 64, 512, 256, 32
HH, WW = 160, 160
HW = HH * WW           # 25600
NCORES = 8
BPC = B // NCORES      # 8 batches per core
NCH = N // 128         # 4 entity chunks of 128
F32 = mybir.dt.float32
BF16 = mybir.dt.bfloat16
I32 = mybir.dt.int32

# comb matmul dtype: "fp32" (exact) or "bf16split" (hi/lo split, ~1e-5 rel)
COMB_MODE = "bf16split"

_NC_CACHE = None


def build_nc():
    nc = bacc.Bacc("TRN2", target_bir_lowering=False, debug=False, num_devices=NCORES)

    embT = nc.dram_tensor("embT", [BPC, D_IN, N], F32, kind="ExternalInput")
    xc = nc.dram_tensor("xc", [BPC, 128, NCH], I32, kind="ExternalInput")
    yc = nc.dram_tensor("yc", [BPC, 128, NCH], I32, kind="ExternalInput")
    xr = nc.dram_tensor("xr", [BPC, 1, N], I32, kind="ExternalInput")
    yr = nc.dram_tensor("yr", [BPC, 1, N], I32, kind="ExternalInput")
    wgt = nc.dram_tensor("wgt", [D_IN, D_OUT], F32, kind="ExternalInput")
    bias = nc.dram_tensor("bias", [D_OUT, 1], F32, kind="ExternalInput")
    entn = nc.dram_tensor("entn", [1, BPC], I32, kind="ExternalInput")
    outs = [
        nc.dram_tensor(f"out{b}", [HW, D_OUT], F32, kind="ExternalOutput")
        for b in range(BPC)
    ]

    sel_dt = F32 if COMB_MODE == "fp32" else BF16

    with tile.TileContext(nc) as tc:
        with (
            tc.tile_pool(name="const", bufs=1) as cpool,
            tc.tile_pool(name="io", bufs=3) as iopool,
            tc.tile_pool(name="work", bufs=3) as wpool,
            tc.tile_pool(name="ppool", bufs=2, space="PSUM") as ppool,
            tc.tile_pool(name="ppool1", bufs=2, space="PSUM") as ppool1,
        ):
            # ---- per-core constants ----
            from concourse.masks import make_identity
            id32 = cpool.tile([32, 32], F32, tag="id32")
            make_identity(nc, id32[:])
            id64 = cpool.tile([64, 64], F32, tag="id64")
            make_identity(nc, id64[:])
            id128 = cpool.tile([128, 128], F32, tag="id128")
            make_identity(nc, id128[:])

            w0 = cpool.tile([128, D_OUT], F32, tag="w0")
            w1 = cpool.tile([128, D_OUT], F32, tag="w1")
            nc.sync.dma_start(out=w0[:], in_=wgt[0:128, :])
            nc.sync.dma_start(out=w1[:], in_=wgt[128:256, :])
            bias_c = cpool.tile([D_OUT, 1], F32, tag="bias_c")
            nc.sync.dma_start(out=bias_c[:], in_=bias[:, :])

            # entity_num broadcast to all partitions via replicating DMA
            entnb = cpool.tile([128, BPC], I32, tag="entnb")
            nc.gpsimd.dma_start(
                out=entnb[:], in_=entn[:, :].to_broadcast([128, BPC])
            )
            mask_tiles = []
            for c in range(NCH):
                iota_c = cpool.tile([128, BPC], I32, tag=f"iota{c}")
                nc.gpsimd.iota(
                    iota_c[:], pattern=[[0, BPC]], base=c * 128, channel_multiplier=1
                )
                m = cpool.tile([128, BPC], F32, tag=f"mask{c}")
                nc.vector.tensor_tensor(
                    out=m[:], in0=iota_c[:], in1=entnb[:], op=mybir.AluOpType.is_lt
                )
                mask_tiles.append(m)

            # ---- per-batch pipeline ----
            for b in range(BPC):
                e0 = iopool.tile([128, N], F32, tag="embT0")
                e1 = iopool.tile([128, N], F32, tag="embT1")
                nc.sync.dma_start(out=e0[:], in_=embT[b, 0:128, :])
                nc.sync.dma_start(out=e1[:], in_=embT[b, 128:256, :])
                xt = iopool.tile([128, NCH], I32, tag="xt")
                yt = iopool.tile([128, NCH], I32, tag="yt")
                nc.sync.dma_start(out=xt[:], in_=xc[b, :, :])
                nc.sync.dma_start(out=yt[:], in_=yc[b, :, :])
                xrt = iopool.tile([1, N], I32, tag="xrt")
                yrt = iopool.tile([1, N], I32, tag="yrt")
                nc.sync.dma_start(out=xrt[:], in_=xr[b, :, :])
                nc.sync.dma_start(out=yrt[:], in_=yr[b, :, :])

                # flat idx, chunk layout [128, 4] (scatter offsets + sel scalars)
                idx_i = wpool.tile([128, NCH], I32, tag="idx_i")
                nc.vector.tensor_scalar(
                    out=idx_i[:], in0=yt[:], scalar1=WW, scalar2=None,
                    op0=mybir.AluOpType.mult,
                )
                nc.vector.tensor_tensor(
                    out=idx_i[:], in0=idx_i[:], in1=xt[:], op=mybir.AluOpType.add
                )
                idx_f = wpool.tile([128, NCH], F32, tag="idx_f")
                nc.vector.tensor_copy(out=idx_f[:], in_=idx_i[:])
                # idx row [1, 512] in entity order, broadcast to all partitions
                idx_row = wpool.tile([1, N], F32, tag="idx_row")
                nc.vector.tensor_scalar(
                    out=idx_row[:], in0=yrt[:], scalar1=WW, scalar2=None,
                    op0=mybir.AluOpType.mult,
                )
                nc.vector.tensor_tensor(
                    out=idx_row[:], in0=idx_row[:], in1=xrt[:], op=mybir.AluOpType.add
                )
                row_ps = ppool1.tile([128, N], F32, tag="row_ps")
                for c in range(NCH):
                    nc.tensor.transpose(
                        out=row_ps[:, c * 128:(c + 1) * 128],
                        in_=idx_f[:, c:c + 1].to_broadcast([128, 128]),
                        identity=id128[:],
                    )
                row_sb = wpool.tile([128, N], F32, tag="row_sb")
                nc.vector.tensor_copy(out=row_sb[:], in_=row_ps[:])

                # selection tiles with mask folded in:
                # sel_c[p, i] = (idx[c*128+p] == idx[i]) * (c*128+p < entity_num)
                sels = []
                for c in range(NCH):
                    s = wpool.tile([128, N], sel_dt, tag=f"sel{c}")
                    nc.vector.tensor_scalar(
                        out=s[:], in0=row_sb[:], scalar1=idx_f[:, c:c + 1],
                        scalar2=mask_tiles[c][:, b:b + 1],
                        op0=mybir.AluOpType.is_equal, op1=mybir.AluOpType.mult,
                    )
                    sels.append(s)

                # projT = relu(W.T @ embT + bias)  [32, 512]
                projT_ps = ppool.tile([32, N], F32, tag="big32_ps")
                nc.tensor.matmul(out=projT_ps[:], lhsT=w0[:], rhs=e0[:],
                                 start=True, stop=False)
                nc.tensor.matmul(out=projT_ps[:], lhsT=w1[:], rhs=e1[:],
                                 start=False, stop=True)
                projT_sb = wpool.tile([32, N], F32, tag="projT_sb")
                nc.scalar.activation(
                    out=projT_sb[:], in_=projT_ps[:],
                    func=mybir.ActivationFunctionType.Relu, bias=bias_c[:, :1],
                )

                # entity-major proj chunks [128, 4*32] via PE transposes
                proj_ps = ppool.tile([128, 128], F32, tag="entmaj_ps")
                for c in range(NCH):
                    nc.tensor.transpose(
                        out=proj_ps[:, c * D_OUT:(c + 1) * D_OUT],
                        in_=projT_sb[:, c * 128:(c + 1) * 128],
                        identity=id32[:],
                    )
                if COMB_MODE == "fp32":
                    proj_sb = wpool.tile([128, 128], F32, tag="proj_sb")
                    for c in range(NCH):
                        sl = slice(c * D_OUT, (c + 1) * D_OUT)
                        nc.scalar.copy(out=proj_sb[:, sl], in_=proj_ps[:, sl])
                    proj_parts = [(proj_sb, 1.0)]
                else:
                    hilo = wpool.tile([128, 2 * 128], BF16, tag="proj_hilo")
                    hilo3 = hilo[:].rearrange("p (c t o) -> p c t o", t=2, o=D_OUT)
                    fsb = wpool.tile([128, 128], F32, tag="proj_f")
                    nc.scalar.copy(out=fsb[:], in_=proj_ps[:])
                    fsb3 = fsb[:].rearrange("p (c o) -> p c o", o=D_OUT)
                    hi3 = hilo3[:, :, 0, :]
                    lo3 = hilo3[:, :, 1, :]
                    nc.vector.tensor_copy(out=hi3, in_=fsb3)
                    nc.vector.tensor_tensor(
                        out=lo3, in0=fsb3, in1=hi3, op=mybir.AluOpType.subtract
                    )

                # group totals, transposed, hi and lo stacked: rows 0:32 hi,
                # rows 32:64 lo -- combT2[2o, i] = sum_j hilo[j, 2o] sel[j, i]
                combT_ps = ppool.tile([64, N], F32, tag="big32_ps")
                for jc in range(NCH):
                    nc.tensor.matmul(
                        out=combT_ps[:],
                        lhsT=hilo[:, jc * 64:(jc + 1) * 64],
                        rhs=sels[jc][:],
                        start=(jc == 0), stop=(jc == NCH - 1),
                    )
                combT_sb = wpool.tile([64, N], F32, tag="combT_sb")
                nc.vector.tensor_copy(out=combT_sb[:], in_=combT_ps[:])

                # back to entity-major and recombine hi+lo while copying out
                comb_ps = ppool.tile([128, 2 * 128], F32, tag="entmaj_ps")
                for c in range(NCH):
                    nc.tensor.transpose(
                        out=comb_ps[:, c * 64:(c + 1) * 64],
                        in_=combT_sb[:, c * 128:(c + 1) * 128],
                        identity=id64[:],
                    )
                comb_sb = wpool.tile([128, 128], F32, tag="comb_sb")
                cp3 = comb_ps[:].rearrange("p (c t o) -> p c t o", t=2, o=D_OUT)
                cs3 = comb_sb[:].rearrange("p (c o) -> p c o", o=D_OUT)
                nc.vector.tensor_copy(out=cs3, in_=cp3[:, :, 0, :])
                nc.vector.tensor_tensor(
                    out=cs3, in0=cs3, in1=cp3[:, :, 1, :], op=mybir.AluOpType.add,
                )

                for c in range(NCH):
                    nc.gpsimd.indirect_dma_start(
                        out=outs[b][:, :],
                        out_offset=bass.IndirectOffsetOnAxis(
                            ap=idx_i[:, c:c + 1], axis=0
                        ),
                        in_=comb_sb[:, c * D_OUT:(c + 1) * D_OUT],
                        in_offset=None,
                    )
    nc.compile()
    return nc


def _get_nc():
    global _NC_CACHE
    if _NC_CACHE is None:
        _NC_CACHE = build_nc()
    return _NC_CACHE


def _prep_in_maps(entity_embeddings, weight, bias, entity_x, entity_y, entity_num):
    emb = np.ascontiguousarray(np.asarray(entity_embeddings, dtype=np.float32))
    x = np.asarray(entity_x, dtype=np.int32)
    y = np.asarray(entity_y, dtype=np.int32)
    en = np.asarray(entity_num, dtype=np.int32)
    wgt = np.ascontiguousarray(np.asarray(weight, dtype=np.float32))
    bia = np.ascontiguousarray(np.asarray(bias, dtype=np.float32).reshape(D_OUT, 1))

    embT = np.ascontiguousarray(emb.transpose(0, 2, 1))          # [B, 256, 512]
    xc = np.ascontiguousarray(x.reshape(B, NCH, 128).transpose(0, 2, 1))
    yc = np.ascontiguousarray(y.reshape(B, NCH, 128).transpose(0, 2, 1))
    xrow = np.ascontiguousarray(x.reshape(B, 1, N))
    yrow = np.ascontiguousarray(y.reshape(B, 1, N))

    in_maps = []
    for core in range(NCORES):
        sl = slice(core * BPC, (core + 1) * BPC)
        in_maps.append({
            "embT": embT[sl],
            "xc": xc[sl],
            "yc": yc[sl],
            "xr": xrow[sl],
            "yr": yrow[sl],
            "wgt": wgt,
            "bias": bia,
            "entn": en[sl].reshape(1, BPC),
        })
    return in_maps


def run(inputs, trace=False, trace_cores=None):
    """Run the bass kernel; returns (full_output [B,32,H,W], BassKernelResults)."""
    nc = _get_nc()
    in_maps = _prep_in_maps(
        inputs["entity_embeddings"], inputs["weight"], inputs["bias"],
        inputs["entity_x"], inputs["entity_y"], inputs["entity_num"],
    )
    res = run_bass_kernel_spmd(
        nc, in_maps, core_ids=list(range(NCORES)), trace=trace,
        trace_cores=trace_cores,
    )
    dev = np.empty((B, HW, D_OUT), dtype=np.float32)
    for core in range(NCORES):
        for b in range(BPC):
            dev[core * BPC + b] = res.results[core][f"out{b}"]
    full = np.ascontiguousarray(
        dev.reshape(B, HH, WW, D_OUT).transpose(0, 3, 1, 2)
    )
    return full, res


def kernel(**inputs) -> np.ndarray:
    full, _ = run(inputs, trace=False)
    return full


# revision 17
# speedup vs baseline: 1.0703x; 1.0703x over previous
"""Trainium2 Bass kernel for nn_Encoder_88235808129468 (scatter_memory).

reference semantics:
    proj = relu(emb @ W + b)                      # [B, N, 32]
    proj *= (n < entity_num[b])                   # mask padded entities
    out[b, :, y, x] += proj[b, n, :]              # scatter-add into [B, 32, H, W]

Strategy (pure data-parallel over batch: 8 cores x 8 batches):
  - ExternalOutput DRAM buffers are pre-zeroed by the PJRT runner (documented
    contract in bass2jax.run_bass_via_pjrt), so the kernel only writes the
    scattered entity rows; untouched cells stay zero.
  - Device output layout is HW-major [25600, 32] rows per batch; the host
    reorders axes to [B, 32, H, W] at the end.
  - Scatter-add duplicates are resolved with a 512x512 selection matrix
    (idx_i == idx_j, with the padding mask folded in): a matmul against proj
    gives EVERY entity the full group total for its cell, then an overwrite
    indirect-DMA scatter writes the rows. Colliding writes carry identical
    bytes, so write order does not matter.
  - proj and the group totals are computed in transposed [32, N] space so
    the matmuls run at F=512 with few instructions, then PE-transposed back
    to entity-major tiles for the scatter.
"""
import sys
import types

sys.path.insert(0, "/opt/trn_rl_repo")

import numpy as np


def _install_axon_hooks_stub():
    """bass_utils imports antenv.axon_hooks when tracing; give it a no-op."""
    if "antenv.axon_hooks" in sys.modules:
        return
    mod = types.ModuleType("antenv.axon_hooks")
    _state = {"hook": None}
    mod.set_axon_ntff_profile_hook = lambda h: _state.__setitem__("hook", h)
    mod.get_axon_ntff_profile_hook = lambda: _state["hook"]
    sys.modules["antenv.axon_hooks"] = mod


_install_axon_hooks_stub()

from concourse import bass, mybir, bacc  # noqa: E402
from concourse.bass_utils import run_bass_kernel_spmd  # noqa: E402
import concourse.tile as tile  # noqa: E402

# Problem constants (hardcoded per harness contract)
B, N, D_IN, D_OUT = 64, 512, 256, 32
HH, WW = 160, 160
HW = HH * WW           # 25600
NCORES = 8
BPC = B // NCORES      # 8 batches per core
NCH = N // 128         # 4 entity chunks of 128
F32 = mybir.dt.float32
BF16 = mybir.dt.bfloat16
I32 = mybir.dt.int32

# comb matmul dtype: "fp32" (exact) or "bf16split" (hi/lo split, ~1e-5 rel)
COMB_MODE = "bf16split"

_NC_CACHE = None


def build_nc():
    nc = bacc.Bacc("TRN2", target_bir_lowering=False, debug=False, num_devices=NCORES)

    embT = nc.dram_tensor("embT", [BPC, D_IN, N], F32, kind="ExternalInput")
    xc = nc.dram_tensor("xc", [BPC, 128, NCH], I32, kind="ExternalInput")
    yc = nc.dram_tensor("yc", [BPC, 128, NCH], I32, kind="ExternalInput")
    xr = nc.dram_tensor("xr", [BPC, 1, N], I32, kind="ExternalInput")
    yr = nc.dram_tensor("yr", [BPC, 1, N], I32, kind="ExternalInput")
    wgt = nc.dram_tensor("wgt", [D_IN, D_OUT], F32, kind="ExternalInput")
    bias = nc.dram_tensor("bias", [D_OUT, 1], F32, kind="ExternalInput")
    entn = nc.dram_tensor("entn", [1, BPC], I32, kind="ExternalInput")
    outs = [
        nc.dram_tensor(f"out{b}", [HW, D_OUT], F32, kind="ExternalOutput")
        for b in range(BPC)
    ]

    sel_dt = F32 if COMB_MODE == "fp32" else BF16

    with tile.TileContext(nc) as tc:
        with (
            tc.tile_pool(name="const", bufs=1) as cpool,
            tc.tile_pool(name="io", bufs=3) as iopool,
            tc.tile_pool(name="work", bufs=3) as wpool,
            tc.tile_pool(name="ppool", bufs=2, space="PSUM") as ppool,
            tc.tile_pool(name="ppool1", bufs=2, space="PSUM") as ppool1,
        ):
            # ---- per-core constants ----
            from concourse.masks import make_identity
            id32 = cpool.tile([32, 32], F32, tag="id32")
            make_identity(nc, id32[:])
            id64 = cpool.tile([64, 64], F32, tag="id64")
            make_identity(nc, id64[:])
            id128 = cpool.tile([128, 128], F32, tag="id128")
            make_identity(nc, id128[:])

            w0 = cpool.tile([128, D_OUT], F32, tag="w0")
            w1 = cpool.tile([128, D_OUT], F32, tag="w1")
            nc.sync.dma_start(out=w0[:], in_=wgt[0:128, :])
            nc.sync.dma_start(out=w1[:], in_=wgt[128:256, :])
            bias_c = cpool.tile([D_OUT, 1], F32, tag="bias_c")
            nc.sync.dma_start(out=bias_c[:], in_=bias[:, :])

            # entity_num broadcast to all partitions via replicating DMA
            entnb = cpool.tile([128, BPC], I32, tag="entnb")
            nc.gpsimd.dma_start(
                out=entnb[:], in_=entn[:, :].to_broadcast([128, BPC])
            )
            mask_tiles = []
            for c in range(NCH):
                iota_c = cpool.tile([128, BPC], I32, tag=f"iota{c}")
                nc.gpsimd.iota(
                    iota_c[:], pattern=[[0, BPC]], base=c * 128, channel_multiplier=1
                )
                m = cpool.tile([128, BPC], F32, tag=f"mask{c}")
                nc.vector.tensor_tensor(
                    out=m[:], in0=iota_c[:], in1=entnb[:], op=mybir.AluOpType.is_lt
                )
                mask_tiles.append(m)

            # ---- per-batch pipeline ----
            for b in range(BPC):
                e0 = iopool.tile([128, N], F32, tag="embT0")
                e1 = iopool.tile([128, N], F32, tag="embT1")
                nc.sync.dma_start(out=e0[:], in_=embT[b, 0:128, :])
                nc.sync.dma_start(out=e1[:], in_=embT[b, 128:256, :])
                xt = iopool.tile([128, NCH], I32, tag="xt")
                yt = iopool.tile([128, NCH], I32, tag="yt")
                nc.sync.dma_start(out=xt[:], in_=xc[b, :, :])
                nc.sync.dma_start(out=yt[:], in_=yc[b, :, :])

                # flat idx, chunk layout [128, 4] (scatter offsets + sel scalars)
                idx_i = wpool.tile([128, NCH], I32, tag="idx_i")
                nc.vector.tensor_scalar(
                    out=idx_i[:], in0=yt[:], scalar1=WW, scalar2=None,
                    op0=mybir.AluOpType.mult,
                )
                nc.vector.tensor_tensor(
                    out=idx_i[:], in0=idx_i[:], in1=xt[:], op=mybir.AluOpType.add
                )
                idx_f = wpool.tile([128, NCH], F32, tag="idx_f")
                nc.vector.tensor_copy(out=idx_f[:], in_=idx_i[:])
                row_ps = ppool1.tile([128, N], F32, tag="row_ps")
                for c in range(NCH):
                    nc.tensor.transpose(
                        out=row_ps[:, c * 128:(c + 1) * 128],
                        in_=idx_f[:, c:c + 1].to_broadcast([128, 128]),
                        identity=id128[:],
                    )
                row_sb = wpool.tile([128, N], F32, tag="row_sb")
                nc.vector.tensor_copy(out=row_sb[:], in_=row_ps[:])

                # selection tiles with mask folded in:
                # sel_c[p, i] = (idx[c*128+p] == idx[i]) * (c*128+p < entity_num)
                sels = []
                for c in range(NCH):
                    s = wpool.tile([128, N], sel_dt, tag=f"sel{c}")
                    nc.vector.tensor_scalar(
                        out=s[:], in0=row_sb[:], scalar1=idx_f[:, c:c + 1],
                        scalar2=mask_tiles[c][:, b:b + 1],
                        op0=mybir.AluOpType.is_equal, op1=mybir.AluOpType.mult,
                    )
                    sels.append(s)

                # projT = relu(W.T @ embT + bias)  [32, 512]
                projT_ps = ppool.tile([32, N], F32, tag="big32_ps")
                nc.tensor.matmul(out=projT_ps[:], lhsT=w0[:], rhs=e0[:],
                                 start=True, stop=False)
                nc.tensor.matmul(out=projT_ps[:], lhsT=w1[:], rhs=e1[:],
                                 start=False, stop=True)
                projT_sb = wpool.tile([32, N], F32, tag="projT_sb")
                nc.scalar.activation(
                    out=projT_sb[:], in_=projT_ps[:],
                    func=mybir.ActivationFunctionType.Relu, bias=bias_c[:, :1],
                )

                # entity-major proj chunks [128, 4*32] via PE transposes
                proj_ps = ppool.tile([128, 128], F32, tag="entmaj_ps")
                for c in range(NCH):
                    nc.tensor.transpose(
                        out=proj_ps[:, c * D_OUT:(c + 1) * D_OUT],
                        in_=projT_sb[:, c * 128:(c + 1) * 128],
                        identity=id32[:],
                    )
                if COMB_MODE == "fp32":
                    proj_sb = wpool.tile([128, 128], F32, tag="proj_sb")
                    for c in range(NCH):
                        sl = slice(c * D_OUT, (c + 1) * D_OUT)
                        nc.scalar.copy(out=proj_sb[:, sl], in_=proj_ps[:, sl])
                    proj_parts = [(proj_sb, 1.0)]
                else:
                    hilo = wpool.tile([128, 2 * 128], BF16, tag="proj_hilo")
                    hilo3 = hilo[:].rearrange("p (c t o) -> p c t o", t=2, o=D_OUT)
                    fsb = wpool.tile([128, 128], F32, tag="proj_f")
                    nc.scalar.copy(out=fsb[:], in_=proj_ps[:])
                    fsb3 = fsb[:].rearrange("p (c o) -> p c o", o=D_OUT)
                    hi3 = hilo3[:, :, 0, :]
                    lo3 = hilo3[:, :, 1, :]
                    nc.vector.tensor_copy(out=hi3, in_=fsb3)
                    nc.vector.tensor_tensor(
                        out=lo3, in0=fsb3, in1=hi3, op=mybir.AluOpType.subtract
                    )

                # group totals, transposed, hi and lo stacked: rows 0:32 hi,
                # rows 32:64 lo -- combT2[2o, i] = sum_j hilo[j, 2o] sel[j, i]
                combT_ps = ppool.tile([64, N], F32, tag="big32_ps")
                for jc in range(NCH):
                    nc.tensor.matmul(
                        out=combT_ps[:],
                        lhsT=hilo[:, jc * 64:(jc + 1) * 64],
                        rhs=sels[jc][:],
                        start=(jc == 0), stop=(jc == NCH - 1),
                    )
                combT_sb = wpool.tile([64, N], F32, tag="combT_sb")
                nc.vector.tensor_copy(out=combT_sb[:], in_=combT_ps[:])

                # back to entity-major and recombine hi+lo while copying out
                comb_ps = ppool.tile([128, 2 * 128], F32, tag="entmaj_ps")
                for c in range(NCH):
                    nc.tensor.transpose(
                        out=comb_ps[:, c * 64:(c + 1) * 64],
                        in_=combT_sb[:, c * 128:(c + 1) * 128],
                        identity=id64[:],
                    )
                comb_sb = wpool.tile([128, 128], F32, tag="comb_sb")
                cp3 = comb_ps[:].rearrange("p (c t o) -> p c t o", t=2, o=D_OUT)
                cs3 = comb_sb[:].rearrange("p (c o) -> p c o", o=D_OUT)
                nc.vector.tensor_copy(out=cs3, in_=cp3[:, :, 0, :])
                nc.vector.tensor_tensor(
                    out=cs3, in0=cs3, in1=cp3[:, :, 1, :], op=mybir.AluOpType.add,
                )

                for c in range(NCH):
                    nc.gpsimd.indirect_dma_start(
                        out=outs[b][:, :],
                        out_offset=bass.IndirectOffsetOnAxis(
                            ap=idx_i[:, c:c + 1], axis=0
                        ),
                        in_=comb_sb[:, c * D_OUT:(c + 1) * D_OUT],
                        in_offset=None,
                    )
    nc.compile()
    return nc


def _get_nc():
    global _NC_CACHE
    if _NC_CACHE is None:
        _NC_CACHE = build_nc()
    return _NC_CACHE


def _prep_in_maps(entity_embeddings, weight, bias, entity_x, entity_y, entity_num):
    emb = np.ascontiguousarray(np.asarray(entity_embeddings, dtype=np.float32))
    x = np.asarray(entity_x, dtype=np.int32)
    y = np.asarray(entity_y, dtype=np.int32)
    en = np.asarray(entity_num, dtype=np.int32)
    wgt = np.ascontiguousarray(np.asarray(weight, dtype=np.float32))
    bia = np.ascontiguousarray(np.asarray(bias, dtype=np.float32).reshape(D_OUT, 1))

    embT = np.ascontiguousarray(emb.transpose(0, 2, 1))          # [B, 256, 512]
    xc = np.ascontiguousarray(x.reshape(B, NCH, 128).transpose(0, 2, 1))
    yc = np.ascontiguousarray(y.reshape(B, NCH, 128).transpose(0, 2, 1))
    xrow = np.ascontiguousarray(x.reshape(B, 1, N))
    yrow = np.ascontiguousarray(y.reshape(B, 1, N))

    in_maps = []
    for core in range(NCORES):
        sl = slice(core * BPC, (core + 1) * BPC)
        in_maps.append({
            "embT": embT[sl],
            "xc": xc[sl],
            "yc": yc[sl],
            "xr": xrow[sl],
            "yr": yrow[sl],
            "wgt": wgt,
            "bias": bia,
            "entn": en[sl].reshape(1, BPC),
        })
    return in_maps


def run(inputs, trace=False, trace_cores=None):
    """Run the bass kernel; returns (full_output [B,32,H,W], BassKernelResults)."""
    nc = _get_nc()
    in_maps = _prep_in_maps(
        inputs["entity_embeddings"], inputs["weight"], inputs["bias"],
        inputs["entity_x"], inputs["entity_y"], inputs["entity_num"],
    )
    res = run_bass_kernel_spmd(
        nc, in_maps, core_ids=list(range(NCORES)), trace=trace,
        trace_cores=trace_cores,
    )
    dev = np.empty((B, HW, D_OUT), dtype=np.float32)
    for core in range(NCORES):
        for b in range(BPC):
            dev[core * BPC + b] = res.results[core][f"out{b}"]
    full = np.ascontiguousarray(
        dev.reshape(B, HH, WW, D_OUT).transpose(0, 3, 1, 2)
    )
    return full, res


def kernel(**inputs) -> np.ndarray:
    full, _ = run(inputs, trace=False)
    return full


# revision 19
# speedup vs baseline: 1.1655x; 1.0890x over previous
"""Trainium2 Bass kernel for nn_Encoder_88235808129468 (scatter_memory).

reference semantics:
    proj = relu(emb @ W + b)                      # [B, N, 32]
    proj *= (n < entity_num[b])                   # mask padded entities
    out[b, :, y, x] += proj[b, n, :]              # scatter-add into [B, 32, H, W]

Strategy (pure data-parallel over batch: 8 cores x 8 batches):
  - ExternalOutput DRAM buffers are pre-zeroed by the PJRT runner (documented
    contract in bass2jax.run_bass_via_pjrt), so the kernel only writes the
    scattered entity rows; untouched cells stay zero.
  - Device output layout is HW-major [25600, 32] rows per batch; the host
    reorders axes to [B, 32, H, W] at the end.
  - Scatter-add duplicates are resolved with a 512x512 selection matrix
    (idx_i == idx_j, with the padding mask folded in): a matmul against proj
    gives EVERY entity the full group total for its cell, then an overwrite
    indirect-DMA scatter writes the rows. Colliding writes carry identical
    bytes, so write order does not matter.
  - proj and the group totals are computed in transposed [32, N] space so
    the matmuls run at F=512 with few instructions, then PE-transposed back
    to entity-major tiles for the scatter.
"""
import sys
import types

sys.path.insert(0, "/opt/trn_rl_repo")

import numpy as np


def _install_axon_hooks_stub():
    """bass_utils imports antenv.axon_hooks when tracing; give it a no-op."""
    if "antenv.axon_hooks" in sys.modules:
        return
    mod = types.ModuleType("antenv.axon_hooks")
    _state = {"hook": None}
    mod.set_axon_ntff_profile_hook = lambda h: _state.__setitem__("hook", h)
    mod.get_axon_ntff_profile_hook = lambda: _state["hook"]
    sys.modules["antenv.axon_hooks"] = mod


_install_axon_hooks_stub()

from concourse import bass, mybir, bacc  # noqa: E402
from concourse.bass_utils import run_bass_kernel_spmd  # noqa: E402
import concourse.tile as tile  # noqa: E402

# Problem constants (hardcoded per harness contract)
B, N, D_IN, D_OUT = 64, 512, 256, 32
HH, WW = 160, 160
HW = HH * WW           # 25600
NCORES = 8
BPC = B // NCORES      # 8 batches per core
NCH = N // 128         # 4 entity chunks of 128
F32 = mybir.dt.float32
BF16 = mybir.dt.bfloat16
I32 = mybir.dt.int32

# comb matmul dtype: "fp32" (exact) or "bf16split" (hi/lo split, ~1e-5 rel)
COMB_MODE = "bf16split"

_NC_CACHE = None


def build_nc():
    nc = bacc.Bacc("TRN2", target_bir_lowering=False, debug=False, num_devices=NCORES)

    embT = nc.dram_tensor("embT", [BPC, D_IN, N], F32, kind="ExternalInput")
    xc = nc.dram_tensor("xc", [BPC, 128, NCH], I32, kind="ExternalInput")
    yc = nc.dram_tensor("yc", [BPC, 128, NCH], I32, kind="ExternalInput")
    wgt = nc.dram_tensor("wgt", [D_IN, D_OUT], F32, kind="ExternalInput")
    bias = nc.dram_tensor("bias", [D_OUT, 1], F32, kind="ExternalInput")
    entn = nc.dram_tensor("entn", [1, BPC], I32, kind="ExternalInput")
    iota4 = nc.dram_tensor("iota4", [128, NCH], F32, kind="ExternalInput")
    outs = [
        nc.dram_tensor(f"out{b}", [HW, D_OUT], F32, kind="ExternalOutput")
        for b in range(BPC)
    ]

    sel_dt = F32 if COMB_MODE == "fp32" else BF16

    with tile.TileContext(nc) as tc:
        with (
            tc.tile_pool(name="const", bufs=1) as cpool,
            tc.tile_pool(name="io", bufs=3) as iopool,
            tc.tile_pool(name="work", bufs=3) as wpool,
            tc.tile_pool(name="ppool", bufs=2, space="PSUM") as ppool,
            tc.tile_pool(name="ppool1", bufs=2, space="PSUM") as ppool1,
        ):
            # ---- per-core constants ----
            from concourse.masks import make_identity
            id32 = cpool.tile([32, 32], F32, tag="id32")
            make_identity(nc, id32[:])
            id64 = cpool.tile([64, 64], F32, tag="id64")
            make_identity(nc, id64[:])
            id128 = cpool.tile([128, 128], F32, tag="id128")
            make_identity(nc, id128[:])

            w0 = cpool.tile([128, D_OUT], F32, tag="w0")
            w1 = cpool.tile([128, D_OUT], F32, tag="w1")
            nc.sync.dma_start(out=w0[:], in_=wgt[0:128, :])
            nc.sync.dma_start(out=w1[:], in_=wgt[128:256, :])
            bias_c = cpool.tile([D_OUT, 1], F32, tag="bias_c")
            nc.sync.dma_start(out=bias_c[:], in_=bias[:, :])

            # entity_num broadcast to all partitions via replicating DMA
            entnb = cpool.tile([128, BPC], I32, tag="entnb")
            nc.gpsimd.dma_start(
                out=entnb[:], in_=entn[:, :].to_broadcast([128, BPC])
            )
            entnb_f = cpool.tile([128, BPC], F32, tag="entnb_f")
            nc.vector.tensor_copy(out=entnb_f[:], in_=entnb[:])
            iota4_t = cpool.tile([128, NCH], F32, tag="iota4_t")
            nc.sync.dma_start(out=iota4_t[:], in_=iota4[:, :])

            # ---- per-batch pipeline ----
            for b in range(BPC):
                e0 = iopool.tile([128, N], F32, tag="embT0")
                e1 = iopool.tile([128, N], F32, tag="embT1")
                nc.sync.dma_start(out=e0[:], in_=embT[b, 0:128, :])
                nc.sync.dma_start(out=e1[:], in_=embT[b, 128:256, :])
                xt = iopool.tile([128, NCH], I32, tag="xt")
                yt = iopool.tile([128, NCH], I32, tag="yt")
                nc.sync.dma_start(out=xt[:], in_=xc[b, :, :])
                nc.sync.dma_start(out=yt[:], in_=yc[b, :, :])

                # flat idx, chunk layout [128, 4] (scatter offsets + sel scalars)
                idx_i = wpool.tile([128, NCH], I32, tag="idx_i")
                nc.vector.tensor_scalar(
                    out=idx_i[:], in0=yt[:], scalar1=WW, scalar2=None,
                    op0=mybir.AluOpType.mult,
                )
                nc.vector.tensor_tensor(
                    out=idx_i[:], in0=idx_i[:], in1=xt[:], op=mybir.AluOpType.add
                )
                idx_f = wpool.tile([128, NCH], F32, tag="idx_f")
                nc.vector.tensor_copy(out=idx_f[:], in_=idx_i[:])
                # per-batch mask [128, NCH]: (c*128+p < entity_num[b])
                mask_b = wpool.tile([128, NCH], F32, tag="mask_b")
                nc.vector.tensor_scalar(
                    out=mask_b[:], in0=iota4_t[:], scalar1=entnb_f[:, b:b + 1],
                    scalar2=None, op0=mybir.AluOpType.is_lt,
                )
                row_ps = ppool1.tile([128, N], F32, tag="row_ps")
                for c in range(NCH):
                    nc.tensor.transpose(
                        out=row_ps[:, c * 128:(c + 1) * 128],
                        in_=idx_f[:, c:c + 1].to_broadcast([128, 128]),
                        identity=id128[:],
                    )
                row_sb = wpool.tile([128, N], F32, tag="row_sb")
                nc.vector.tensor_copy(out=row_sb[:], in_=row_ps[:])

                # selection tiles with mask folded in:
                # sel_c[p, i] = (idx[c*128+p] == idx[i]) * (c*128+p < entity_num)
                sels = []
                for c in range(NCH):
                    s = wpool.tile([128, N], sel_dt, tag=f"sel{c}")
                    nc.vector.tensor_scalar(
                        out=s[:], in0=row_sb[:], scalar1=idx_f[:, c:c + 1],
                        scalar2=mask_b[:, c:c + 1],
                        op0=mybir.AluOpType.is_equal, op1=mybir.AluOpType.mult,
                    )
                    sels.append(s)

                # projT = relu(W.T @ embT + bias)  [32, 512]
                projT_ps = ppool.tile([32, N], F32, tag="big32_ps")
                nc.tensor.matmul(out=projT_ps[:], lhsT=w0[:], rhs=e0[:],
                                 start=True, stop=False)
                nc.tensor.matmul(out=projT_ps[:], lhsT=w1[:], rhs=e1[:],
                                 start=False, stop=True)
                projT_sb = wpool.tile([32, N], F32, tag="projT_sb")
                nc.scalar.activation(
                    out=projT_sb[:], in_=projT_ps[:],
                    func=mybir.ActivationFunctionType.Relu, bias=bias_c[:, :1],
                )

                # entity-major proj chunks [128, 4*32] via PE transposes
                proj_ps = ppool.tile([128, 128], F32, tag="entmaj_ps")
                for c in range(NCH):
                    nc.tensor.transpose(
                        out=proj_ps[:, c * D_OUT:(c + 1) * D_OUT],
                        in_=projT_sb[:, c * 128:(c + 1) * 128],
                        identity=id32[:],
                    )
                if COMB_MODE == "fp32":
                    proj_sb = wpool.tile([128, 128], F32, tag="proj_sb")
                    for c in range(NCH):
                        sl = slice(c * D_OUT, (c + 1) * D_OUT)
                        nc.scalar.copy(out=proj_sb[:, sl], in_=proj_ps[:, sl])
                    proj_parts = [(proj_sb, 1.0)]
                else:
                    hilo = wpool.tile([128, 2 * 128], BF16, tag="proj_hilo")
                    hilo3 = hilo[:].rearrange("p (c t o) -> p c t o", t=2, o=D_OUT)
                    fsb = wpool.tile([128, 128], F32, tag="proj_f")
                    nc.scalar.copy(out=fsb[:], in_=proj_ps[:])
                    fsb3 = fsb[:].rearrange("p (c o) -> p c o", o=D_OUT)
                    hi3 = hilo3[:, :, 0, :]
                    lo3 = hilo3[:, :, 1, :]
                    nc.vector.tensor_copy(out=hi3, in_=fsb3)
                    nc.vector.tensor_tensor(
                        out=lo3, in0=fsb3, in1=hi3, op=mybir.AluOpType.subtract
                    )

                # group totals, transposed, hi and lo stacked: rows 0:32 hi,
                # rows 32:64 lo -- combT2[2o, i] = sum_j hilo[j, 2o] sel[j, i]
                combT_ps = ppool.tile([64, N], F32, tag="big32_ps")
                for jc in range(NCH):
                    nc.tensor.matmul(
                        out=combT_ps[:],
                        lhsT=hilo[:, jc * 64:(jc + 1) * 64],
                        rhs=sels[jc][:],
                        start=(jc == 0), stop=(jc == NCH - 1),
                    )
                combT_sb = wpool.tile([64, N], F32, tag="combT_sb")
                nc.vector.tensor_copy(out=combT_sb[:], in_=combT_ps[:])

                # back to entity-major and recombine hi+lo while copying out
                comb_ps = ppool.tile([128, 2 * 128], F32, tag="entmaj_ps")
                for c in range(NCH):
                    nc.tensor.transpose(
                        out=comb_ps[:, c * 64:(c + 1) * 64],
                        in_=combT_sb[:, c * 128:(c + 1) * 128],
                        identity=id64[:],
                    )
                comb_sb = wpool.tile([128, 128], F32, tag="comb_sb")
                cp3 = comb_ps[:].rearrange("p (c t o) -> p c t o", t=2, o=D_OUT)
                cs3 = comb_sb[:].rearrange("p (c o) -> p c o", o=D_OUT)
                nc.vector.tensor_copy(out=cs3, in_=cp3[:, :, 0, :])
                nc.vector.tensor_tensor(
                    out=cs3, in0=cs3, in1=cp3[:, :, 1, :], op=mybir.AluOpType.add,
                )

                # masked entities -> idx + 2^20 (beyond bounds, skipped)
                big = wpool.tile([128, NCH], F32, tag="big")
                nc.vector.tensor_scalar(
                    out=big[:], in0=mask_b[:], scalar1=-float(2 ** 20),
                    scalar2=float(2 ** 20), op0=mybir.AluOpType.mult,
                    op1=mybir.AluOpType.add,
                )
                idx_sc = wpool.tile([128, NCH], I32, tag="idx_sc")
                nc.vector.tensor_tensor(
                    out=idx_sc[:], in0=idx_f[:], in1=big[:], op=mybir.AluOpType.add
                )
                for c in range(NCH):
                    nc.gpsimd.indirect_dma_start(
                        out=outs[b][:, :],
                        out_offset=bass.IndirectOffsetOnAxis(
                            ap=idx_sc[:, c:c + 1], axis=0
                        ),
                        in_=comb_sb[:, c * D_OUT:(c + 1) * D_OUT],
                        in_offset=None,
                        bounds_check=HW - 1,
                        oob_is_err=False,
                    )
    nc.compile()
    return nc


def _get_nc():
    global _NC_CACHE
    if _NC_CACHE is None:
        _NC_CACHE = build_nc()
    return _NC_CACHE


def _prep_in_maps(entity_embeddings, weight, bias, entity_x, entity_y, entity_num):
    emb = np.ascontiguousarray(np.asarray(entity_embeddings, dtype=np.float32))
    x = np.asarray(entity_x, dtype=np.int32)
    y = np.asarray(entity_y, dtype=np.int32)
    en = np.asarray(entity_num, dtype=np.int32)
    wgt = np.ascontiguousarray(np.asarray(weight, dtype=np.float32))
    bia = np.ascontiguousarray(np.asarray(bias, dtype=np.float32).reshape(D_OUT, 1))

    embT = np.ascontiguousarray(emb.transpose(0, 2, 1))          # [B, 256, 512]
    xc = np.ascontiguousarray(x.reshape(B, NCH, 128).transpose(0, 2, 1))
    yc = np.ascontiguousarray(y.reshape(B, NCH, 128).transpose(0, 2, 1))

    iota4_arr = np.ascontiguousarray(
        (np.arange(128, dtype=np.float32)[:, None]
         + 128.0 * np.arange(NCH, dtype=np.float32)[None, :])
    )
    in_maps = []
    for core in range(NCORES):
        sl = slice(core * BPC, (core + 1) * BPC)
        in_maps.append({
            "embT": embT[sl],
            "xc": xc[sl],
            "yc": yc[sl],
            "wgt": wgt,
            "bias": bia,
            "entn": en[sl].reshape(1, BPC),
            "iota4": iota4_arr,
        })
    return in_maps


def run(inputs, trace=False, trace_cores=None):
    """Run the bass kernel; returns (full_output [B,32,H,W], BassKernelResults)."""
    nc = _get_nc()
    in_maps = _prep_in_maps(
        inputs["entity_embeddings"], inputs["weight"], inputs["bias"],
        inputs["entity_x"], inputs["entity_y"], inputs["entity_num"],
    )
    res = run_bass_kernel_spmd(
        nc, in_maps, core_ids=list(range(NCORES)), trace=trace,
        trace_cores=trace_cores,
    )
    dev = np.empty((B, HW, D_OUT), dtype=np.float32)
    for core in range(NCORES):
        for b in range(BPC):
            dev[core * BPC + b] = res.results[core][f"out{b}"]
    full = np.ascontiguousarray(
        dev.reshape(B, HH, WW, D_OUT).transpose(0, 3, 1, 2)
    )
    return full, res


def kernel(**inputs) -> np.ndarray:
    full, _ = run(inputs, trace=False)
    return full


# revision 21
# speedup vs baseline: 1.1930x; 1.0236x over previous
"""Trainium2 Bass kernel for nn_Encoder_88235808129468 (scatter_memory).

reference semantics:
    proj = relu(emb @ W + b)                      # [B, N, 32]
    proj *= (n < entity_num[b])                   # mask padded entities
    out[b, :, y, x] += proj[b, n, :]              # scatter-add into [B, 32, H, W]

Strategy (pure data-parallel over batch: 8 cores x 8 batches):
  - ExternalOutput DRAM buffers are pre-zeroed by the PJRT runner (documented
    contract in bass2jax.run_bass_via_pjrt), so the kernel only writes the
    scattered entity rows; untouched cells stay zero.
  - Device output layout is HW-major [25600, 32] rows per batch; the host
    reorders axes to [B, 32, H, W] at the end.
  - Scatter-add duplicates are resolved with a 512x512 selection matrix
    (idx_i == idx_j, padding mask folded in): a matmul against proj gives
    EVERY entity the full group total for its cell, then an overwrite
    indirect-DMA scatter writes the rows; colliding writes carry identical
    bytes so write order does not matter. Padded entities are skipped via
    bounds_check (their index is pushed out of range).
  - Heavy math runs in transposed [32/64, N] space with two batches fused
    per pass; values use bf16 hi+lo splitting (exact to ~2^-16) so every
    matmul is single-pass bf16; PE transposes return to entity-major.
  - Embeddings arrive as host-split bf16 hi/lo pairs (same total bytes as
    fp32, lossless to 16 mantissa bits).
"""
import sys
import types

sys.path.insert(0, "/opt/trn_rl_repo")

import numpy as np


def _install_axon_hooks_stub():
    """bass_utils imports antenv.axon_hooks when tracing; give it a no-op."""
    if "antenv.axon_hooks" in sys.modules:
        return
    mod = types.ModuleType("antenv.axon_hooks")
    _state = {"hook": None}
    mod.set_axon_ntff_profile_hook = lambda h: _state.__setitem__("hook", h)
    mod.get_axon_ntff_profile_hook = lambda: _state["hook"]
    sys.modules["antenv.axon_hooks"] = mod


_install_axon_hooks_stub()

from concourse import bass, mybir, bacc  # noqa: E402
from concourse.bass_utils import run_bass_kernel_spmd  # noqa: E402
import concourse.tile as tile  # noqa: E402

B, N, D_IN, D_OUT = 64, 512, 256, 32
HH, WW = 160, 160
HW = HH * WW           # 25600
NCORES = 8
BPC = B // NCORES      # 8 batches per core
NCH = N // 128         # 4 entity chunks of 128
F32 = mybir.dt.float32
BF16 = mybir.dt.bfloat16
I32 = mybir.dt.int32

_NC_CACHE = None


def build_nc():
    nc = bacc.Bacc("TRN2", target_bir_lowering=False, debug=False, num_devices=NCORES)

    ehi = nc.dram_tensor("ehi", [BPC, D_IN, N], BF16, kind="ExternalInput")
    elo = nc.dram_tensor("elo", [BPC, D_IN, N], BF16, kind="ExternalInput")
    xc = nc.dram_tensor("xc", [BPC, 128, NCH], I32, kind="ExternalInput")
    yc = nc.dram_tensor("yc", [BPC, 128, NCH], I32, kind="ExternalInput")
    wgt = nc.dram_tensor("wgt", [D_IN, D_OUT], F32, kind="ExternalInput")
    bias = nc.dram_tensor("bias", [D_OUT, 1], F32, kind="ExternalInput")
    entn = nc.dram_tensor("entn", [1, BPC], I32, kind="ExternalInput")
    iota4 = nc.dram_tensor("iota4", [128, NCH], F32, kind="ExternalInput")
    outs = [
        nc.dram_tensor(f"out{b}", [HW, D_OUT], F32, kind="ExternalOutput")
        for b in range(BPC)
    ]

    with tile.TileContext(nc) as tc:
        with (
            tc.tile_pool(name="const", bufs=1) as cpool,
            tc.tile_pool(name="io", bufs=3) as iopool,
            tc.tile_pool(name="work", bufs=3) as wpool,
            tc.tile_pool(name="ppool", bufs=2, space="PSUM") as ppool,
        ):
            # ---- per-core constants ----
            from concourse.masks import make_identity
            id32 = cpool.tile([32, 32], F32, tag="id32")
            make_identity(nc, id32[:])
            id128 = cpool.tile([128, 128], F32, tag="id128")
            make_identity(nc, id128[:])

            wf = cpool.tile([128, 2 * D_OUT], F32, tag="wf")
            nc.sync.dma_start(out=wf[:, :D_OUT], in_=wgt[0:128, :])
            nc.sync.dma_start(out=wf[:, D_OUT:], in_=wgt[128:256, :])
            whi = cpool.tile([128, 2 * D_OUT], BF16, tag="whi")
            wlo = cpool.tile([128, 2 * D_OUT], BF16, tag="wlo")
            nc.vector.tensor_copy(out=whi[:], in_=wf[:])
            nc.vector.tensor_tensor(
                out=wlo[:], in0=wf[:], in1=whi[:], op=mybir.AluOpType.subtract
            )
            bias_c = cpool.tile([D_OUT, 1], F32, tag="bias_c")
            nc.sync.dma_start(out=bias_c[:], in_=bias[:, :])

            entnb = cpool.tile([128, BPC], I32, tag="entnb")
            nc.gpsimd.dma_start(
                out=entnb[:], in_=entn[:, :].to_broadcast([128, BPC])
            )
            entnb_f = cpool.tile([128, BPC], F32, tag="entnb_f")
            nc.vector.tensor_copy(out=entnb_f[:], in_=entnb[:])
            iota4_t = cpool.tile([128, NCH], F32, tag="iota4_t")
            nc.sync.dma_start(out=iota4_t[:], in_=iota4[:, :])

            # ---- per-pair pipeline (two batches per pass) ----
            for pb in range(BPC // 2):
                bb = (2 * pb, 2 * pb + 1)
                ehi_t, elo_t = [], []
                for t, b in enumerate(bb):
                    eh = iopool.tile([128, 2 * N], BF16, tag=f"ehi{t}")
                    el = iopool.tile([128, 2 * N], BF16, tag=f"elo{t}")
                    nc.sync.dma_start(out=eh[:, :N], in_=ehi[b, 0:128, :])
                    nc.sync.dma_start(out=eh[:, N:], in_=ehi[b, 128:256, :])
                    nc.sync.dma_start(out=el[:, :N], in_=elo[b, 0:128, :])
                    nc.sync.dma_start(out=el[:, N:], in_=elo[b, 128:256, :])
                    ehi_t.append(eh)
                    elo_t.append(el)

                idx_f2, mask2, sels2, idx_sc2 = [], [], [], []
                for t, b in enumerate(bb):
                    xt = iopool.tile([128, NCH], I32, tag=f"xt{t}")
                    yt = iopool.tile([128, NCH], I32, tag=f"yt{t}")
                    nc.sync.dma_start(out=xt[:], in_=xc[b, :, :])
                    nc.sync.dma_start(out=yt[:], in_=yc[b, :, :])
                    idx_i = wpool.tile([128, NCH], I32, tag=f"idx_i{t}")
                    nc.vector.tensor_scalar(
                        out=idx_i[:], in0=yt[:], scalar1=WW, scalar2=None,
                        op0=mybir.AluOpType.mult,
                    )
                    nc.vector.tensor_tensor(
                        out=idx_i[:], in0=idx_i[:], in1=xt[:],
                        op=mybir.AluOpType.add,
                    )
                    idx_f = wpool.tile([128, NCH], F32, tag=f"idx_f{t}")
                    nc.vector.tensor_copy(out=idx_f[:], in_=idx_i[:])
                    idx_f2.append(idx_f)
                    mask_b = wpool.tile([128, NCH], F32, tag=f"mask_b{t}")
                    nc.vector.tensor_scalar(
                        out=mask_b[:], in0=iota4_t[:],
                        scalar1=entnb_f[:, b:b + 1], scalar2=None,
                        op0=mybir.AluOpType.is_lt,
                    )
                    mask2.append(mask_b)
                    # masked entities -> out-of-range idx (scatter skips them)
                    big = wpool.tile([128, NCH], F32, tag=f"big{t}")
                    nc.vector.tensor_scalar(
                        out=big[:], in0=mask_b[:], scalar1=-float(2 ** 20),
                        scalar2=float(2 ** 20), op0=mybir.AluOpType.mult,
                        op1=mybir.AluOpType.add,
                    )
                    idx_sc = wpool.tile([128, NCH], I32, tag=f"idx_sc{t}")
                    nc.vector.tensor_tensor(
                        out=idx_sc[:], in0=idx_f[:], in1=big[:],
                        op=mybir.AluOpType.add,
                    )
                    idx_sc2.append(idx_sc)

                    # idx broadcast row + selection tiles (bf16, mask folded)
                    row_ps = ppool.tile([128, N], F32, tag="row_ps")
                    for c in range(NCH):
                        nc.tensor.transpose(
                            out=row_ps[:, c * 128:(c + 1) * 128],
                            in_=idx_f[:, c:c + 1].to_broadcast([128, 128]),
                            identity=id128[:],
                        )
                    row_sb = wpool.tile([128, N], F32, tag=f"row_sb{t}")
                    nc.vector.tensor_copy(out=row_sb[:], in_=row_ps[:])
                    sels = []
                    for c in range(NCH):
                        s = wpool.tile([128, N], BF16, tag=f"sel{t}_{c}")
                        nc.vector.tensor_scalar(
                            out=s[:], in0=row_sb[:], scalar1=idx_f[:, c:c + 1],
                            scalar2=mask_b[:, c:c + 1],
                            op0=mybir.AluOpType.is_equal,
                            op1=mybir.AluOpType.mult,
                        )
                        sels.append(s)
                    sels2.append(sels)

                # projT pair [32, 1024]: cols t*N.. hold batch t
                # = whi.T(ehi+elo) + wlo.T ehi   (lo*lo dropped, ~2^-16)
                projT_ps = ppool.tile([32, 2 * N], F32, tag="projT_ps")
                for t in range(2):
                    osl = slice(t * N, (t + 1) * N)
                    for k in range(2):
                        ksl = slice(k * N, (k + 1) * N)
                        wsl = slice(k * D_OUT, (k + 1) * D_OUT)
                        nc.tensor.matmul(
                            out=projT_ps[:, osl], lhsT=whi[:, wsl],
                            rhs=ehi_t[t][:, ksl], start=(k == 0), stop=False,
                        )
                        nc.tensor.matmul(
                            out=projT_ps[:, osl], lhsT=whi[:, wsl],
                            rhs=elo_t[t][:, ksl], start=False, stop=False,
                        )
                        nc.tensor.matmul(
                            out=projT_ps[:, osl], lhsT=wlo[:, wsl],
                            rhs=ehi_t[t][:, ksl], start=False,
                            stop=(k == 1),
                        )
                projT_sb = wpool.tile([32, 2 * N], F32, tag="projT_sb")
                nc.scalar.activation(
                    out=projT_sb[:], in_=projT_ps[:],
                    func=mybir.ActivationFunctionType.Relu, bias=bias_c[:, :1],
                )

                # entity-major proj [128, (c t o)] via 8 PE transposes
                proj_ps = ppool.tile([128, 2 * 128], F32, tag="entmaj_ps")
                for t in range(2):
                    for c in range(NCH):
                        nc.tensor.transpose(
                            out=proj_ps[:, (c * 2 + t) * D_OUT:
                                        (c * 2 + t + 1) * D_OUT],
                            in_=projT_sb[:, t * N + c * 128:
                                         t * N + (c + 1) * 128],
                            identity=id32[:],
                        )
                fsb = wpool.tile([128, 2 * 128], F32, tag="proj_f")
                nc.scalar.copy(out=fsb[:], in_=proj_ps[:])
                # hi/lo split, laid out [c][t][hl][o] so comb lhsT slices are
                # contiguous [hi|lo] blocks per (chunk, batch)
                hilo = wpool.tile([128, 2 * 256], BF16, tag="proj_hilo")
                hl5 = hilo[:].rearrange("p (c t h o) -> p c t h o", t=2, h=2,
                                        o=D_OUT)
                f4 = fsb[:].rearrange("p (c t o) -> p c t o", t=2, o=D_OUT)
                hi4 = hl5[:, :, :, 0, :]
                lo4 = hl5[:, :, :, 1, :]
                nc.vector.tensor_copy(out=hi4, in_=f4)
                nc.vector.tensor_tensor(
                    out=lo4, in0=f4, in1=hi4, op=mybir.AluOpType.subtract
                )

                # group totals, partition-paired: rows t*64.. = batch t (hi|lo)
                combT_ps = ppool.tile([128, N], F32, tag="projT_ps")
                for t in range(2):
                    psl = slice(t * 64, (t + 1) * 64)
                    for jc in range(NCH):
                        nc.tensor.matmul(
                            out=combT_ps[psl, :],
                            lhsT=hilo[:, (jc * 2 + t) * 64:
                                      (jc * 2 + t + 1) * 64],
                            rhs=sels2[t][jc][:],
                            start=(jc == 0), stop=(jc == NCH - 1),
                        )
                combT_sb = wpool.tile([128, N], F32, tag="combT_sb")
                nc.scalar.copy(out=combT_sb[:], in_=combT_ps[:])

                # back to entity-major: 4 transposes [128,128]; free layout
                # per chunk: [b0hi(32) b0lo(32) b1hi(32) b1lo(32)]
                comb_ps = ppool.tile([128, 2 * 256], F32, tag="entmaj_ps")
                for c in range(NCH):
                    nc.tensor.transpose(
                        out=comb_ps[:, c * 128:(c + 1) * 128],
                        in_=combT_sb[:, c * 128:(c + 1) * 128],
                        identity=id128[:],
                    )
                comb_sb = wpool.tile([128, 2 * 128], F32, tag="comb_sb")
                cp5 = comb_ps[:].rearrange("p (c t h o) -> p c t h o", t=2,
                                           h=2, o=D_OUT)
                cs4 = comb_sb[:].rearrange("p (c t o) -> p c t o", t=2,
                                           o=D_OUT)
                nc.vector.tensor_copy(out=cs4, in_=cp5[:, :, :, 0, :])
                nc.vector.tensor_tensor(
                    out=cs4, in0=cs4, in1=cp5[:, :, :, 1, :],
                    op=mybir.AluOpType.add,
                )

                for t, b in enumerate(bb):
                    for c in range(NCH):
                        nc.gpsimd.indirect_dma_start(
                            out=outs[b][:, :],
                            out_offset=bass.IndirectOffsetOnAxis(
                                ap=idx_sc2[t][:, c:c + 1], axis=0
                            ),
                            in_=comb_sb[:, (c * 2 + t) * D_OUT:
                                        (c * 2 + t + 1) * D_OUT],
                            in_offset=None,
                            bounds_check=HW - 1,
                            oob_is_err=False,
                        )
    nc.compile()
    return nc


def _get_nc():
    global _NC_CACHE
    if _NC_CACHE is None:
        _NC_CACHE = build_nc()
    return _NC_CACHE


def _prep_in_maps(entity_embeddings, weight, bias, entity_x, entity_y, entity_num):
    import ml_dtypes
    emb = np.asarray(entity_embeddings, dtype=np.float32)
    x = np.asarray(entity_x, dtype=np.int32)
    y = np.asarray(entity_y, dtype=np.int32)
    en = np.asarray(entity_num, dtype=np.int32)
    wgt = np.ascontiguousarray(np.asarray(weight, dtype=np.float32))
    bia = np.ascontiguousarray(np.asarray(bias, dtype=np.float32).reshape(D_OUT, 1))

    embT = np.ascontiguousarray(emb.transpose(0, 2, 1))          # [B, 256, 512]
    ehi = embT.astype(ml_dtypes.bfloat16)
    elo = (embT - ehi.astype(np.float32)).astype(ml_dtypes.bfloat16)
    xc = np.ascontiguousarray(x.reshape(B, NCH, 128).transpose(0, 2, 1))
    yc = np.ascontiguousarray(y.reshape(B, NCH, 128).transpose(0, 2, 1))
    iota4_arr = np.ascontiguousarray(
        np.arange(128, dtype=np.float32)[:, None]
        + 128.0 * np.arange(NCH, dtype=np.float32)[None, :]
    )

    in_maps = []
    for core in range(NCORES):
        sl = slice(core * BPC, (core + 1) * BPC)
        in_maps.append({
            "ehi": ehi[sl],
            "elo": elo[sl],
            "xc": xc[sl],
            "yc": yc[sl],
            "wgt": wgt,
            "bias": bia,
            "entn": en[sl].reshape(1, BPC),
            "iota4": iota4_arr,
        })
    return in_maps


def run(inputs, trace=False, trace_cores=None):
    """Run the bass kernel; returns (full_output [B,32,H,W], BassKernelResults)."""
    nc = _get_nc()
    in_maps = _prep_in_maps(
        inputs["entity_embeddings"], inputs["weight"], inputs["bias"],
        inputs["entity_x"], inputs["entity_y"], inputs["entity_num"],
    )
    res = run_bass_kernel_spmd(
        nc, in_maps, core_ids=list(range(NCORES)), trace=trace,
        trace_cores=trace_cores,
    )
    dev = np.empty((B, HW, D_OUT), dtype=np.float32)
    for core in range(NCORES):
        for b in range(BPC):
            dev[core * BPC + b] = res.results[core][f"out{b}"]
    full = np.ascontiguousarray(
        dev.reshape(B, HH, WW, D_OUT).transpose(0, 3, 1, 2)
    )
    return full, res


def kernel(**inputs) -> np.ndarray:
    full, _ = run(inputs, trace=False)
    return full


# revision 23
# speedup vs baseline: 1.2049x; 1.0100x over previous
"""Trainium2 Bass kernel for nn_Encoder_88235808129468 (scatter_memory).

reference semantics:
    proj = relu(emb @ W + b)                      # [B, N, 32]
    proj *= (n < entity_num[b])                   # mask padded entities
    out[b, :, y, x] += proj[b, n, :]              # scatter-add into [B, 32, H, W]

Strategy (pure data-parallel over batch: 8 cores x 8 batches):
  - ExternalOutput DRAM buffers are pre-zeroed by the PJRT runner (documented
    contract in bass2jax.run_bass_via_pjrt), so the kernel only writes the
    scattered entity rows; untouched cells stay zero.
  - Device output layout is HW-major [25600, 32] rows per batch; the host
    reorders axes to [B, 32, H, W] at the end.
  - Scatter-add duplicates are resolved with a 512x512 selection matrix
    (idx_i == idx_j, padding mask folded in): a matmul against proj gives
    EVERY entity the full group total for its cell, then an overwrite
    indirect-DMA scatter writes the rows; colliding writes carry identical
    bytes so write order does not matter. Padded entities are skipped via
    bounds_check (their index is pushed out of range).
  - Heavy math runs in transposed [32/64, N] space with two batches fused
    per pass; values use bf16 hi+lo splitting (exact to ~2^-16) so every
    matmul is single-pass bf16; PE transposes return to entity-major.
  - Embeddings arrive as host-split bf16 hi/lo pairs (same total bytes as
    fp32, lossless to 16 mantissa bits).
"""
import sys
import types

sys.path.insert(0, "/opt/trn_rl_repo")

import numpy as np


def _install_axon_hooks_stub():
    """bass_utils imports antenv.axon_hooks when tracing; give it a no-op."""
    if "antenv.axon_hooks" in sys.modules:
        return
    mod = types.ModuleType("antenv.axon_hooks")
    _state = {"hook": None}
    mod.set_axon_ntff_profile_hook = lambda h: _state.__setitem__("hook", h)
    mod.get_axon_ntff_profile_hook = lambda: _state["hook"]
    sys.modules["antenv.axon_hooks"] = mod


_install_axon_hooks_stub()

from concourse import bass, mybir, bacc  # noqa: E402
from concourse.bass_utils import run_bass_kernel_spmd  # noqa: E402
import concourse.tile as tile  # noqa: E402

B, N, D_IN, D_OUT = 64, 512, 256, 32
HH, WW = 160, 160
HW = HH * WW           # 25600
NCORES = 8
BPC = B // NCORES      # 8 batches per core
NCH = N // 128         # 4 entity chunks of 128
F32 = mybir.dt.float32
BF16 = mybir.dt.bfloat16
I32 = mybir.dt.int32

_NC_CACHE = None


def build_nc():
    nc = bacc.Bacc("TRN2", target_bir_lowering=False, debug=False, num_devices=NCORES)

    ehi = nc.dram_tensor("ehi", [BPC, D_IN, N], BF16, kind="ExternalInput")
    elo = nc.dram_tensor("elo", [BPC, D_IN, N], BF16, kind="ExternalInput")
    xc = nc.dram_tensor("xc", [BPC, 128, NCH], I32, kind="ExternalInput")
    yc = nc.dram_tensor("yc", [BPC, 128, NCH], I32, kind="ExternalInput")
    wgt = nc.dram_tensor("wgt", [D_IN, D_OUT], F32, kind="ExternalInput")
    bias = nc.dram_tensor("bias", [D_OUT, 1], F32, kind="ExternalInput")
    entn = nc.dram_tensor("entn", [1, BPC], I32, kind="ExternalInput")
    iota4 = nc.dram_tensor("iota4", [128, NCH], F32, kind="ExternalInput")
    outs = [
        nc.dram_tensor(f"out{b}", [HW, D_OUT], F32, kind="ExternalOutput")
        for b in range(BPC)
    ]

    with tile.TileContext(nc) as tc:
        with (
            tc.tile_pool(name="const", bufs=1) as cpool,
            tc.tile_pool(name="io", bufs=4) as iopool,
            tc.tile_pool(name="work", bufs=4) as wpool,
            tc.tile_pool(name="ppool", bufs=2, space="PSUM") as ppool,
        ):
            # ---- per-core constants ----
            from concourse.masks import make_identity
            id32 = cpool.tile([32, 32], F32, tag="id32")
            make_identity(nc, id32[:])
            id128 = cpool.tile([128, 128], F32, tag="id128")
            make_identity(nc, id128[:])

            wf = cpool.tile([128, 2 * D_OUT], F32, tag="wf")
            nc.sync.dma_start(out=wf[:, :D_OUT], in_=wgt[0:128, :])
            nc.sync.dma_start(out=wf[:, D_OUT:], in_=wgt[128:256, :])
            whi = cpool.tile([128, 2 * D_OUT], BF16, tag="whi")
            wlo = cpool.tile([128, 2 * D_OUT], BF16, tag="wlo")
            nc.vector.tensor_copy(out=whi[:], in_=wf[:])
            nc.vector.tensor_tensor(
                out=wlo[:], in0=wf[:], in1=whi[:], op=mybir.AluOpType.subtract
            )
            bias_c = cpool.tile([D_OUT, 1], F32, tag="bias_c")
            nc.sync.dma_start(out=bias_c[:], in_=bias[:, :])

            entnb = cpool.tile([128, BPC], I32, tag="entnb")
            nc.gpsimd.dma_start(
                out=entnb[:], in_=entn[:, :].to_broadcast([128, BPC])
            )
            entnb_f = cpool.tile([128, BPC], F32, tag="entnb_f")
            nc.vector.tensor_copy(out=entnb_f[:], in_=entnb[:])
            iota4_t = cpool.tile([128, NCH], F32, tag="iota4_t")
            nc.sync.dma_start(out=iota4_t[:], in_=iota4[:, :])

            # ---- per-pair pipeline (two batches per pass) ----
            for pb in range(BPC // 2):
                bb = (2 * pb, 2 * pb + 1)
                ehk, elk = [], []
                for k in range(2):
                    eh = iopool.tile([128, 2 * N], BF16, tag=f"ehk{k}")
                    el = iopool.tile([128, 2 * N], BF16, tag=f"elk{k}")
                    for t, b in enumerate(bb):
                        nc.sync.dma_start(
                            out=eh[:, t * N:(t + 1) * N],
                            in_=ehi[b, k * 128:(k + 1) * 128, :],
                        )
                        nc.sync.dma_start(
                            out=el[:, t * N:(t + 1) * N],
                            in_=elo[b, k * 128:(k + 1) * 128, :],
                        )
                    ehk.append(eh)
                    elk.append(el)

                idx_f2, mask2, sels2, idx_sc2 = [], [], [], []
                for t, b in enumerate(bb):
                    xt = iopool.tile([128, NCH], I32, tag=f"xt{t}")
                    yt = iopool.tile([128, NCH], I32, tag=f"yt{t}")
                    nc.sync.dma_start(out=xt[:], in_=xc[b, :, :])
                    nc.sync.dma_start(out=yt[:], in_=yc[b, :, :])
                    idx_i = wpool.tile([128, NCH], I32, tag=f"idx_i{t}")
                    nc.vector.tensor_scalar(
                        out=idx_i[:], in0=yt[:], scalar1=WW, scalar2=None,
                        op0=mybir.AluOpType.mult,
                    )
                    nc.vector.tensor_tensor(
                        out=idx_i[:], in0=idx_i[:], in1=xt[:],
                        op=mybir.AluOpType.add,
                    )
                    idx_f = wpool.tile([128, NCH], F32, tag=f"idx_f{t}")
                    nc.vector.tensor_copy(out=idx_f[:], in_=idx_i[:])
                    idx_f2.append(idx_f)
                    mask_b = wpool.tile([128, NCH], F32, tag=f"mask_b{t}")
                    nc.vector.tensor_scalar(
                        out=mask_b[:], in0=iota4_t[:],
                        scalar1=entnb_f[:, b:b + 1], scalar2=None,
                        op0=mybir.AluOpType.is_lt,
                    )
                    mask2.append(mask_b)
                    # masked entities -> out-of-range idx (scatter skips them)
                    big = wpool.tile([128, NCH], F32, tag=f"big{t}")
                    nc.vector.tensor_scalar(
                        out=big[:], in0=mask_b[:], scalar1=-float(2 ** 20),
                        scalar2=float(2 ** 20), op0=mybir.AluOpType.mult,
                        op1=mybir.AluOpType.add,
                    )
                    idx_sc = wpool.tile([128, NCH], I32, tag=f"idx_sc{t}")
                    nc.vector.tensor_tensor(
                        out=idx_sc[:], in0=idx_f[:], in1=big[:],
                        op=mybir.AluOpType.add,
                    )
                    idx_sc2.append(idx_sc)

                    # idx broadcast row + selection tiles (bf16, mask folded)
                    row_ps = ppool.tile([128, N], F32, tag="row_ps")
                    for c in range(NCH):
                        nc.tensor.transpose(
                            out=row_ps[:, c * 128:(c + 1) * 128],
                            in_=idx_f[:, c:c + 1].to_broadcast([128, 128]),
                            identity=id128[:],
                        )
                    row_sb = wpool.tile([128, N], F32, tag=f"row_sb{t}")
                    nc.vector.tensor_copy(out=row_sb[:], in_=row_ps[:])
                    sels = []
                    for c in range(NCH):
                        s = wpool.tile([128, N], BF16, tag=f"sel{t}_{c}")
                        nc.vector.tensor_scalar(
                            out=s[:], in0=row_sb[:], scalar1=idx_f[:, c:c + 1],
                            scalar2=mask_b[:, c:c + 1],
                            op0=mybir.AluOpType.is_equal,
                            op1=mybir.AluOpType.mult,
                        )
                        sels.append(s)
                    sels2.append(sels)

                # projT pair [32, 1024]: cols t*N.. hold batch t
                # = whi.T(ehi+elo) + wlo.T ehi   (lo*lo dropped, ~2^-16)
                projT_ps = ppool.tile([32, 2 * N], F32, tag="projT_ps")
                for t in range(2):
                    osl = slice(t * N, (t + 1) * N)
                    tsl = slice(t * N, (t + 1) * N)
                    for k in range(2):
                        wsl = slice(k * D_OUT, (k + 1) * D_OUT)
                        nc.tensor.matmul(
                            out=projT_ps[:, osl], lhsT=whi[:, wsl],
                            rhs=ehk[k][:, tsl], start=(k == 0), stop=False,
                        )
                        nc.tensor.matmul(
                            out=projT_ps[:, osl], lhsT=whi[:, wsl],
                            rhs=elk[k][:, tsl], start=False, stop=False,
                        )
                        nc.tensor.matmul(
                            out=projT_ps[:, osl], lhsT=wlo[:, wsl],
                            rhs=ehk[k][:, tsl], start=False, stop=(k == 1),
                        )
                projT_sb = wpool.tile([32, 2 * N], F32, tag="projT_sb")
                nc.scalar.activation(
                    out=projT_sb[:], in_=projT_ps[:],
                    func=mybir.ActivationFunctionType.Relu, bias=bias_c[:, :1],
                )

                # entity-major proj [128, (c t o)] via 8 PE transposes
                proj_ps = ppool.tile([128, 2 * 128], F32, tag="entmaj_ps")
                for t in range(2):
                    for c in range(NCH):
                        nc.tensor.transpose(
                            out=proj_ps[:, (c * 2 + t) * D_OUT:
                                        (c * 2 + t + 1) * D_OUT],
                            in_=projT_sb[:, t * N + c * 128:
                                         t * N + (c + 1) * 128],
                            identity=id32[:],
                        )
                fsb = wpool.tile([128, 2 * 128], F32, tag="proj_f")
                nc.scalar.copy(out=fsb[:], in_=proj_ps[:])
                # hi/lo split, laid out [c][t][hl][o] so comb lhsT slices are
                # contiguous [hi|lo] blocks per (chunk, batch)
                hilo = wpool.tile([128, 2 * 256], BF16, tag="proj_hilo")
                hl5 = hilo[:].rearrange("p (c t h o) -> p c t h o", t=2, h=2,
                                        o=D_OUT)
                f4 = fsb[:].rearrange("p (c t o) -> p c t o", t=2, o=D_OUT)
                hi4 = hl5[:, :, :, 0, :]
                lo4 = hl5[:, :, :, 1, :]
                nc.vector.tensor_copy(out=hi4, in_=f4)
                nc.vector.tensor_tensor(
                    out=lo4, in0=f4, in1=hi4, op=mybir.AluOpType.subtract
                )

                # group totals, partition-paired: rows t*64.. = batch t (hi|lo)
                combT_ps = ppool.tile([128, N], F32, tag="projT_ps")
                for t in range(2):
                    psl = slice(t * 64, (t + 1) * 64)
                    for jc in range(NCH):
                        nc.tensor.matmul(
                            out=combT_ps[psl, :],
                            lhsT=hilo[:, (jc * 2 + t) * 64:
                                      (jc * 2 + t + 1) * 64],
                            rhs=sels2[t][jc][:],
                            start=(jc == 0), stop=(jc == NCH - 1),
                        )
                combT_sb = wpool.tile([128, N], F32, tag="combT_sb")
                nc.scalar.copy(out=combT_sb[:], in_=combT_ps[:])

                # back to entity-major: 4 transposes [128,128]; free layout
                # per chunk: [b0hi(32) b0lo(32) b1hi(32) b1lo(32)]
                comb_ps = ppool.tile([128, 2 * 256], F32, tag="entmaj_ps")
                for c in range(NCH):
                    nc.tensor.transpose(
                        out=comb_ps[:, c * 128:(c + 1) * 128],
                        in_=combT_sb[:, c * 128:(c + 1) * 128],
                        identity=id128[:],
                    )
                comb_sb = wpool.tile([128, 2 * 128], F32, tag="comb_sb")
                cp5 = comb_ps[:].rearrange("p (c t h o) -> p c t h o", t=2,
                                           h=2, o=D_OUT)
                cs4 = comb_sb[:].rearrange("p (c t o) -> p c t o", t=2,
                                           o=D_OUT)
                nc.vector.tensor_copy(out=cs4, in_=cp5[:, :, :, 0, :])
                nc.vector.tensor_tensor(
                    out=cs4, in0=cs4, in1=cp5[:, :, :, 1, :],
                    op=mybir.AluOpType.add,
                )

                for t, b in enumerate(bb):
                    for c in range(NCH):
                        nc.gpsimd.indirect_dma_start(
                            out=outs[b][:, :],
                            out_offset=bass.IndirectOffsetOnAxis(
                                ap=idx_sc2[t][:, c:c + 1], axis=0
                            ),
                            in_=comb_sb[:, (c * 2 + t) * D_OUT:
                                        (c * 2 + t + 1) * D_OUT],
                            in_offset=None,
                            bounds_check=HW - 1,
                            oob_is_err=False,
                        )
    nc.compile()
    return nc


def _get_nc():
    global _NC_CACHE
    if _NC_CACHE is None:
        _NC_CACHE = build_nc()
    return _NC_CACHE


def _prep_in_maps(entity_embeddings, weight, bias, entity_x, entity_y, entity_num):
    import ml_dtypes
    emb = np.asarray(entity_embeddings, dtype=np.float32)
    x = np.asarray(entity_x, dtype=np.int32)
    y = np.asarray(entity_y, dtype=np.int32)
    en = np.asarray(entity_num, dtype=np.int32)
    wgt = np.ascontiguousarray(np.asarray(weight, dtype=np.float32))
    bia = np.ascontiguousarray(np.asarray(bias, dtype=np.float32).reshape(D_OUT, 1))

    embT = np.ascontiguousarray(emb.transpose(0, 2, 1))          # [B, 256, 512]
    ehi = embT.astype(ml_dtypes.bfloat16)
    elo = (embT - ehi.astype(np.float32)).astype(ml_dtypes.bfloat16)
    xc = np.ascontiguousarray(x.reshape(B, NCH, 128).transpose(0, 2, 1))
    yc = np.ascontiguousarray(y.reshape(B, NCH, 128).transpose(0, 2, 1))
    iota4_arr = np.ascontiguousarray(
        np.arange(128, dtype=np.float32)[:, None]
        + 128.0 * np.arange(NCH, dtype=np.float32)[None, :]
    )

    in_maps = []
    for core in range(NCORES):
        sl = slice(core * BPC, (core + 1) * BPC)
        in_maps.append({
            "ehi": ehi[sl],
            "elo": elo[sl],
            "xc": xc[sl],
            "yc": yc[sl],
            "wgt": wgt,
            "bias": bia,
            "entn": en[sl].reshape(1, BPC),
            "iota4": iota4_arr,
        })
    return in_maps


def run(inputs, trace=False, trace_cores=None):
    """Run the bass kernel; returns (full_output [B,32,H,W], BassKernelResults)."""
    nc = _get_nc()
    in_maps = _prep_in_maps(
        inputs["entity_embeddings"], inputs["weight"], inputs["bias"],
        inputs["entity_x"], inputs["entity_y"], inputs["entity_num"],
    )
    res = run_bass_kernel_spmd(
        nc, in_maps, core_ids=list(range(NCORES)), trace=trace,
        trace_cores=trace_cores,
    )
    dev = np.empty((B, HW, D_OUT), dtype=np.float32)
    for core in range(NCORES):
        for b in range(BPC):
            dev[core * BPC + b] = res.results[core][f"out{b}"]
    full = np.ascontiguousarray(
        dev.reshape(B, HH, WW, D_OUT).transpose(0, 3, 1, 2)
    )
    return full, res


def kernel(**inputs) -> np.ndarray:
    full, _ = run(inputs, trace=False)
    return full
